# revision 1
# baseline (speedup 1.0000x reference)
"""Trainium2 Bass kernel for masked attention + LayerNorm (nn_Attention_4183298146361).

Per-core (data-parallel over batch=8), factorized low-rank formulation:
  scores^T = k_aug^T-slices @ (M^T @ q_aug^T)  with M = [Wq;bq][Wk;bk]^T/16
             (contraction 98 instead of 256 -> one matmul per score tile)
  E = exp(S^T) via 2-bank [128,1024] ACT instructions ping-ponged across two
  PSUM pools (scores of group g+1 fill pool B while ACT drains pool A), then
  masked in place: DVE uint32 bitwise-AND against 0xFFFF/0x0000 words for
  most chunks, gpsimd fp16 multiply against 1.0/0.0 for chunks {1,4,6}
  AVraw^T[c,q] = sum_kt v_aug-tile[kt] @ E^T[kt]  (98-row psum accumulator,
  interleaved one block behind the score stream to keep the PE dense)
  out_aug[q, 0:257] = AVraw^T-slice^T @ Wc  where Wc is host-row-centered so
  the LayerNorm mean is 0 by construction; col 256 = softmax denominator
  LayerNorm: rstd = exp(-0.5*ln(var + eps*denom^2)); o = out*rstd (scale inv.)
"""
import sys

sys.path.insert(0, "/opt/trn_rl_repo")

import numpy as np

import concourse.bacc as bacc
import concourse.tile as tile
from concourse import mybir
from concourse.bass_utils import run_bass_kernel_spmd

# Force a single ACT table set (covers Exp/Ln/Copy) so the table-load pass
# never thrashes between exp_and_others and natural_log_exp_and_others.
_orig_get_tables = bacc.get_activation_tables
def _single_set_tables(arch):
    tabs = _orig_get_tables(arch)
    return {name: (fns if name == "natural_log_exp_and_others" else set())
            for name, fns in tabs.items()}
bacc.get_activation_tables = _single_set_tables

F32 = mybir.dt.float32
F16 = mybir.dt.float16
U16 = mybir.dt.uint16
U32 = mybir.dt.uint32
AF = mybir.ActivationFunctionType
OP = mybir.AluOpType

S = 2048          # sequence length per batch
F = 96            # input feature dim
H = 256           # hidden dim
NCORES = 8
EPS = 1e-6
QB = 512          # q-block width
NBLK = S // QB    # 4
KT = 16           # 2048 / 128 k-tiles
FP = F + 2        # augmented feature rows: 96 + ones row + zero pad
FA = F + 1        # used feature rows (96 + ones)
HO = H + 1        # projection cols: 256 h + denom


def build_nc(identity_gb=False):
    nc = bacc.Bacc()

    qT_d = nc.dram_tensor("qT", [FP, S], F16, kind="ExternalInput")
    kT_d = nc.dram_tensor("kT", [FP, S], F16, kind="ExternalInput")
    vp_d = nc.dram_tensor("vp", [128, KT * FP], F16, kind="ExternalInput")
    # mask bits packed per (q-block, ktgroup): [NBLK, 4, 128, 2048] uint16
    # (0xFFFF keep / 0x0000 drop), 4KB-contiguous per-partition runs
    mask_d = nc.dram_tensor("maskT", [NBLK, 4, 128, 4 * QB], U16, kind="ExternalInput")
    m_d = nc.dram_tensor("mqk", [FP, FP], F16, kind="ExternalInput")
    wc_d = nc.dram_tensor("wc", [FP, HO], F16, kind="ExternalInput")
    gamma_d = nc.dram_tensor("gamma", [H], F32, kind="ExternalInput")
    beta_d = nc.dram_tensor("beta", [H], F32, kind="ExternalInput")
    out_dt = F16 if identity_gb else F32
    out_d = nc.dram_tensor("out", [S, H], out_dt, kind="ExternalOutput")

    with tile.TileContext(nc) as tc:
        with (
            tc.tile_pool(name="consts", bufs=1) as consts,
            tc.tile_pool(name="mask", bufs=8) as maskp,
            tc.tile_pool(name="et", bufs=2) as etp,
            tc.tile_pool(name="fin", bufs=2) as finp,
            tc.tile_pool(name="outp", bufs=3) as outp,
            tc.tile_pool(name="ps_s", bufs=2, space="PSUM") as ps_s,
            tc.tile_pool(name="ps_a", bufs=1, space="PSUM") as ps_a,
            tc.tile_pool(name="ps_p", bufs=3, space="PSUM") as ps_p,
        ):
            # ---- load params via HWDGE ----
            m_sb = consts.tile([FP, FP], F16, name="m_sb", tag="m_sb")
            qT = consts.tile([FP, S], F16, name="qT", tag="qT")
            kTt = consts.tile([FP, S], F16, name="kTt", tag="kTt")
            vp = consts.tile([128, KT * FP], F16, name="vp", tag="vp")
            wc = consts.tile([FP, HO], F16, name="wc", tag="wc")
            nc.sync.dma_start(out=m_sb, in_=m_d[:, :])
            nc.sync.dma_start(out=qT[:, 0:1024], in_=qT_d[:, 0:1024])
            nc.sync.dma_start(out=qT[:, 1024:2048], in_=qT_d[:, 1024:2048])
            nc.sync.dma_start(out=kTt[:, 0:1024], in_=kT_d[:, 0:1024])
            nc.sync.dma_start(out=kTt[:, 1024:2048], in_=kT_d[:, 1024:2048])
            if not identity_gb:
                gam = consts.tile([128, H], F32, name="gam", tag="gam")
                bet = consts.tile([128, H], F32, name="bet", tag="bet")
                nc.sync.dma_start(out=gam, in_=gamma_d[:].partition_broadcast(128))
                nc.sync.dma_start(out=bet, in_=beta_d[:].partition_broadcast(128))

            # mask prefetch state: tiles[blk][g]
            mtiles = {}

            def prefetch_mask(blk, g):
                mk = maskp.tile([128, 4 * QB], U16, name="mk", tag="mk")
                nc.sync.dma_start(out=mk, in_=mask_d[blk, g])
                mtiles[(blk, g)] = mk

            for g in range(4):
                prefetch_mask(0, g)
            # vp/wc are first needed one/two cycles in; don't delay masks
            nc.sync.dma_start(out=vp, in_=vp_d[:, :])
            nc.sync.dma_start(out=wc, in_=wc_d[:, :])

            # ---- QM^T = M^T @ q_aug^T  -> [98, 2048] f16 ----
            QMT = consts.tile([FP, S], F16, name="QMT", tag="QMT")
            for c2 in range(2):
                qps = ps_s.tile([128, 2 * QB], F32, name="sg", tag="sg")
                for h in range(2):
                    c = 2 * c2 + h
                    nc.tensor.matmul(
                        out=qps[0:FP, h * QB:(h + 1) * QB],
                        lhsT=m_sb[:, :],
                        rhs=qT[:, c * QB:(c + 1) * QB],
                        start=True, stop=True,
                    )
                    # chunked evacuation: scores of blk c start after chunk c
                    nc.scalar.activation(
                        out=QMT[:, c * QB:(c + 1) * QB],
                        in_=qps[0:FP, h * QB:(h + 1) * QB], func=AF.Copy)

            ET = [etp.tile([128, KT * QB], F16, name=f"ET{i}", tag=f"ET{i}")
                  for i in range(2)]
            avp = {}

            # half-chunks (2 k-tiles = 1024 wide) masked on gpsimd via fp16
            # multiply; the rest on DVE via uint32 bitwise-AND
            GPS_SET = (1, 4, 6) if identity_gb else ()

            def emit_scores_g(blk, g2):
                sg = ps_s.tile([128, 2 * QB], F32, name="sg", tag="sg")
                for t in range(2):
                    kt = 2 * g2 + t
                    nc.tensor.matmul(
                        out=sg[:, t * QB:(t + 1) * QB],
                        lhsT=kTt[:, kt * 128:(kt + 1) * 128],
                        rhs=QMT[:, blk * QB:(blk + 1) * QB],
                        start=True, stop=True,
                    )
                etc = ET[blk % 2][:, g2 * 2 * QB:(g2 + 1) * 2 * QB]
                nc.scalar.activation(out=etc, in_=sg, func=AF.Exp)
                g, h = divmod(g2, 2)
                mk = mtiles[(blk, g)][:, h * 2 * QB:(h + 1) * 2 * QB]
                if g2 in GPS_SET:
                    nc.gpsimd.tensor_tensor(
                        out=etc, in0=etc, in1=mk.bitcast(F16), op=OP.mult)
                else:
                    nc.vector.tensor_tensor(
                        out=etc.bitcast(U32), in0=etc.bitcast(U32),
                        in1=mk.bitcast(U32), op=OP.bitwise_and)
                if h == 1:
                    mtiles.pop((blk, g))

            def emit_av_g(blk, g2):
                for t in range(2):
                    kt = 2 * g2 + t
                    nc.tensor.matmul(
                        out=avp[blk][0:FP, :],
                        lhsT=vp[:, kt * FP:(kt + 1) * FP],
                        rhs=ET[blk % 2][:, kt * QB:(kt + 1) * QB],
                        start=(kt == 0), stop=(kt == KT - 1),
                    )

            def emit_tail(p):
                avs = finp.tile([128, QB], F16, name="avs", tag="avs")
                nc.vector.tensor_copy(out=avs[0:FP, :], in_=avp.pop(p)[0:FP, :])
                mv = finp.tile([128, 4, 2], F32, name="mv", tag="mv")
                dcol = finp.tile([128, 4], F32, name="dcol", tag="dcol")
                pjcs = []
                for qt in range(4):
                    pj = ps_p.tile([128, HO], F32, name="pj", tag="pj")
                    nc.tensor.matmul(
                        out=pj,
                        lhsT=avs[0:FP, qt * 128:(qt + 1) * 128],
                        rhs=wc[:, :],
                        start=True, stop=True,
                    )
                    # evacuate the bank immediately; stats run from SBUF
                    pjc = finp.tile([128, HO], F16, name="pjc", tag="pjc", bufs=4)
                    nc.vector.tensor_copy(out=pjc, in_=pj)
                    st6 = finp.tile([128, 6], F32, name="st6", tag="st6")
                    nc.vector.bn_stats(out=st6, in_=pjc[:, 0:H])
                    nc.vector.bn_aggr(out=mv[:, qt, :], in_=st6)
                    nc.vector.tensor_copy(out=dcol[:, qt:qt + 1], in_=pjc[:, H:H + 1])
                    pjcs.append(pjc)
                ve = finp.tile([128, 4], F32, name="ve", tag="ve")
                nc.vector.tensor_tensor(out=ve, in0=dcol, in1=dcol, op=OP.mult)
                nc.vector.tensor_scalar_mul(out=ve, in0=ve, scalar1=float(EPS))
                nc.vector.tensor_tensor(out=ve, in0=ve, in1=mv[:, :, 1], op=OP.add)
                rstd = finp.tile([128, 4], F32, name="rstd", tag="rstd")
                nc.scalar.activation(out=rstd, in_=ve, func=AF.Ln)
                nc.scalar.activation(out=rstd, in_=rstd, func=AF.Exp, scale=-0.5)
                for qt, pjc in enumerate(pjcs):
                    o_n = outp.tile([128, H], out_dt, name="o_n", tag="o_n")
                    nc.vector.tensor_scalar_mul(
                        out=o_n, in0=pjc[:, 0:H], scalar1=rstd[:, qt:qt + 1])
                    if not identity_gb:
                        nc.gpsimd.tensor_tensor(out=o_n, in0=o_n, in1=gam, op=OP.mult)
                        nc.gpsimd.tensor_tensor(out=o_n, in0=o_n, in1=bet, op=OP.add)
                    row0 = p * QB + qt * 128
                    nc.sync.dma_start(out=out_d[row0:row0 + 128, :], in_=o_n)

            # ---- main software-pipelined loop ----
            for blk in range(NBLK):
                # tail first: its avs-copy must precede this cycle's ANDs in
                # the DVE queue or AV-psum reuse deadlocks against it
                if blk >= 2:
                    emit_tail(blk - 2)
                if blk >= 1:
                    avp[blk - 1] = ps_a.tile([128, QB], F32, name="av", tag="av")
                if blk == NBLK - 1:
                    avp[blk] = ps_a.tile([128, QB], F32, name="av", tag="av")
                for g2 in range(8):
                    emit_scores_g(blk, g2)
                    if blk + 1 < NBLK and g2 % 2 == 1:
                        prefetch_mask(blk + 1, g2 // 2)
                    if blk >= 1:
                        emit_av_g(blk - 1, g2)
                    if blk == NBLK - 1:
                        # last block: fold its own AV in right behind the mask
                        emit_av_g(blk, g2)
            emit_tail(NBLK - 2)
            emit_tail(NBLK - 1)

    nc.finalize()
    return nc


_NC = {}


def _get_nc(identity_gb=False):
    if identity_gb not in _NC:
        _NC[identity_gb] = build_nc(identity_gb)
    return _NC[identity_gb]


def make_in_maps(query, key, value, mask, Wq, bq, Wk, bk, Wv, bv, gamma, beta):
    B = query.shape[0]
    idgb = bool(np.all(gamma == 1.0) and np.all(beta == 0.0))
    # M = [Wq; bq] @ [Wk; bk]^T / sqrt(H), padded to [98, 98]
    wq_a = np.concatenate([np.asarray(Wq, np.float64),
                           np.asarray(bq, np.float64)[None, :]], 0)
    wk_a = np.concatenate([np.asarray(Wk, np.float64),
                           np.asarray(bk, np.float64)[None, :]], 0)
    m_full = np.zeros((FP, FP), dtype=np.float32)
    m_full[:FA, :FA] = (wq_a @ wk_a.T) / 16.0
    m_full = m_full.astype(np.float16)
    # Wc = row-centered [Wv; bv] plus denominator column
    wv_a = np.concatenate([np.asarray(Wv, np.float64),
                           np.asarray(bv, np.float64)[None, :]], 0)
    wv_c = wv_a - wv_a.mean(axis=1, keepdims=True)
    wc = np.zeros((FP, HO), dtype=np.float32)
    wc[:FA, :H] = wv_c
    wc[F, H] = 1.0
    wc = wc.astype(np.float16)
    gamma = np.ascontiguousarray(np.asarray(gamma, np.float32))
    beta = np.ascontiguousarray(np.asarray(beta, np.float32))

    ones_row = np.ones((1, S), dtype=np.float32)
    zero_row = np.zeros((1, S), dtype=np.float32)
    in_maps = []
    for b in range(B):
        qT = np.concatenate([query[b].T, ones_row, zero_row], 0).astype(np.float16)
        kT = np.concatenate([key[b].T, ones_row, zero_row], 0).astype(np.float16)
        v_aug = np.concatenate(
            [np.asarray(value[b], np.float32),
             np.ones((S, 1), np.float32), np.zeros((S, 1), np.float32)],
            1).astype(np.float16)                                 # [2048, 98]
        vpk = np.ascontiguousarray(
            v_aug.reshape(KT, 128, FP).transpose(1, 0, 2).reshape(128, KT * FP))
        mbits = np.where(mask[b].T != 0, np.uint16(0xFFFF), np.uint16(0))
        # k = g*512 + t*128 + p ; q = blk*QB + qq
        mbits = mbits.reshape(4, 4, 128, NBLK, QB).transpose(3, 0, 2, 1, 4)
        mbits = np.ascontiguousarray(mbits.reshape(NBLK, 4, 128, 4 * QB))
        if idgb:
            # gpsimd-masked half-chunks (g2 in GPS_SET={1,4,6}) use fp16
            # multiply: 1.0/0.0 instead of bit patterns
            one16 = np.float16(1.0).view(np.uint16)
            for g2 in (1, 4, 6):
                g, h = divmod(g2, 2)
                sl = mbits[:, g, :, h * 1024:(h + 1) * 1024]
                sl[...] = np.where(sl != 0, one16, np.uint16(0))
        in_maps.append({
            "qT": np.ascontiguousarray(qT),
            "kT": np.ascontiguousarray(kT),
            "vp": vpk,
            "maskT": mbits,
            "mqk": m_full, "wc": wc,
            "gamma": gamma, "beta": beta,
        })
    return in_maps


def kernel(query, key, value, mask, Wq, bq, Wk, bk, Wv, bv, gamma, beta):
    in_maps = make_in_maps(query, key, value, mask, Wq, bq, Wk, bk, Wv, bv,
                           gamma, beta)
    idgb = bool(np.all(gamma == 1.0) and np.all(beta == 0.0))
    nc = _get_nc(idgb)
    res = run_bass_kernel_spmd(nc, in_maps, list(range(NCORES)))
    out = np.stack([res.results[c]["out"] for c in range(NCORES)], axis=0)
    return out.astype(np.float32)

